# revision 2
# baseline (speedup 1.0000x reference)
"""CrossAttention Trainium2 kernel (Bass/Tile), 8-core SPMD.

Problem: q = query@Wq+bq; k = key@Wk+bk; v = value@Wv+bv;
         out = softmax(q k^T) v           (no 1/sqrt(d) scaling)
Shapes:  query [4, 2048, 1024], key/value [4, 2048, 768],
         W* [(1024|768), 1024], b* [1024], out [4, 2048, 1024] f32.

Sharding: data-parallel over (batch, query-half) -> 8 shards of 1024 query
rows. Each core redundantly projects its batch's full K/V (no collectives).

Precision: projections + scores run the PE in float32r (rounded fp32,
1 cyc/row at N>=512; measured logit abs err ~5e-3 on sigma=32 logits);
softmax probs and V are bf16 for the final GEMM (linear error, ~2^-9).
"""

import os
import sys
from contextlib import ExitStack

for _p in ("/opt/trn_rl_repo", "/root/.axon_site/_ro/trn_rl_repo"):
    if os.path.isdir(_p) and _p not in sys.path:
        sys.path.append(_p)

import numpy as np

import concourse.bass as bass
import concourse.mybir as mybir
import concourse.tile as tile
from concourse import bacc
from concourse.bass import ts
from concourse.bass_utils import run_bass_kernel_spmd
from concourse.masks import make_identity

P = 128
B, LQ, LK = 4, 2048, 2048
D1, D2, H = 1024, 768, 1024
N_CORES = 8
M = (B * LQ) // N_CORES  # 1024 query rows per core

D1T, D2T, HT, MT, JT, JC = D1 // P, D2 // P, H // P, M // P, LK // P, LK // 512

F32 = mybir.dt.float32
F32R = mybir.dt.float32r
BF16 = mybir.dt.bfloat16
AX = mybir.AxisListType.X
AF = mybir.ActivationFunctionType
ALU = mybir.AluOpType

_CACHE = {}
LAST_RESULTS = None  # BassKernelResults of the most recent run (for test harness)


def _build_bass():
    nc = bacc.Bacc("TRN2", target_bir_lowering=False, debug=False,
                   num_devices=N_CORES)

    xq = nc.dram_tensor("xq", [M, D1], F32, kind="ExternalInput")
    ky = nc.dram_tensor("ky", [LK, D2], F32, kind="ExternalInput")
    vv = nc.dram_tensor("vv", [LK, D2], F32, kind="ExternalInput")
    wq = nc.dram_tensor("wq", [D1, H], F32R, kind="ExternalInput")
    wk = nc.dram_tensor("wk", [D2, H], F32R, kind="ExternalInput")
    wv = nc.dram_tensor("wv", [D2, H], F32R, kind="ExternalInput")
    bqd = nc.dram_tensor("bq", [H], F32, kind="ExternalInput")
    bkd = nc.dram_tensor("bk", [H], F32, kind="ExternalInput")
    bvd = nc.dram_tensor("bv", [H], F32, kind="ExternalInput")
    out = nc.dram_tensor("out", [M, H], F32, kind="ExternalOutput")

    with tile.TileContext(nc) as tc, ExitStack() as top:
        const = top.enter_context(tc.tile_pool(name="const", bufs=1))
        ident = const.tile([P, P], F32)
        make_identity(nc, ident[:])
        identb = const.tile([P, P], BF16)
        make_identity(nc, identb[:])
        bqt = const.tile([P, HT], F32)
        nc.sync.dma_start(bqt[:], bqd.rearrange("(t p) -> p t", p=P))
        bkt = const.tile([P, HT], F32)
        nc.sync.dma_start(bkt[:], bkd.rearrange("(t p) -> p t", p=P))
        bv_full = const.tile([P, H], F32)
        nc.sync.dma_start(bv_full[:], bvd[None, :].to_broadcast([P, H]))

        # Residents: qT [H, M], kT [H, LK] (f32r), v [LK, H] (bf16)
        respool = top.enter_context(tc.tile_pool(name="res", bufs=1))
        qT = respool.tile([P, HT, M], F32R)
        kT = respool.tile([P, HT, LK], F32R)

        # ---- Stage A: qT[h, m] = Wq^T @ X^T + bq ----
        with tc.tile_pool(name="sa1", bufs=1) as sa1, \
             tc.tile_pool(name="sa2", bufs=3) as sa2, \
             tc.tile_pool(name="tpsA", bufs=4, space="PSUM") as tpsA, \
             tc.tile_pool(name="ppsA", bufs=2, space="PSUM") as ppsA:
            wqs = sa1.tile([P, D1T, H], F32R)
            nc.sync.dma_start(wqs[:], wq.rearrange("(t p) h -> p t h", p=P))
            xT = sa1.tile([P, D1T, M], F32R)
            for mt in range(MT):
                xrow = sa2.tile([P, D1], F32, tag="xrow")
                nc.sync.dma_start(xrow[:], xq[ts(mt, P), :])
                for dt in range(D1T):
                    pst = tpsA.tile([P, P], F32, tag="pst")
                    nc.tensor.transpose(pst[:], xrow[:, ts(dt, P)], ident[:])
                    nc.vector.tensor_copy(xT[:, dt, ts(mt, P)], pst[:])
            for ht in range(HT):
                for mc in range(M // 512):
                    psq = ppsA.tile([P, 512], F32, tag="psq")
                    for dt in range(D1T):
                        nc.tensor.matmul(psq[:], wqs[:, dt, ts(ht, P)],
                                         xT[:, dt, ts(mc, 512)],
                                         start=(dt == 0), stop=(dt == D1T - 1))
                    nc.scalar.activation(qT[:, ht, ts(mc, 512)], psq[:],
                                         AF.Identity, bias=bqt[:, ht:ht + 1],
                                         scale=1.0)

        # ---- Stage B: kT[h, j] = Wk^T @ Y^T + bk ----
        with tc.tile_pool(name="sb1", bufs=1) as sb1, \
             tc.tile_pool(name="sb2", bufs=3) as sb2, \
             tc.tile_pool(name="sb3", bufs=2) as sb3, \
             tc.tile_pool(name="tpsB", bufs=4, space="PSUM") as tpsB, \
             tc.tile_pool(name="ppsB", bufs=2, space="PSUM") as ppsB:
            wks = sb1.tile([P, D2T, H], F32R)
            nc.sync.dma_start(wks[:], wk.rearrange("(t p) h -> p t h", p=P))
            for jc in range(JC):
                yTc = sb3.tile([P, D2T, 512], F32R, tag="yTc")
                for jt4 in range(4):
                    jt = jc * 4 + jt4
                    yrow = sb2.tile([P, D2], F32, tag="yrow")
                    nc.sync.dma_start(yrow[:], ky[ts(jt, P), :])
                    for dt in range(D2T):
                        pst = tpsB.tile([P, P], F32, tag="pst")
                        nc.tensor.transpose(pst[:], yrow[:, ts(dt, P)], ident[:])
                        nc.vector.tensor_copy(yTc[:, dt, ts(jt4, P)], pst[:])
                for ht in range(HT):
                    psk = ppsB.tile([P, 512], F32, tag="psk")
                    for dt in range(D2T):
                        nc.tensor.matmul(psk[:], wks[:, dt, ts(ht, P)],
                                         yTc[:, dt, :],
                                         start=(dt == 0), stop=(dt == D2T - 1))
                    nc.scalar.activation(kT[:, ht, ts(jc, 512)], psk[:],
                                         AF.Identity, bias=bkt[:, ht:ht + 1],
                                         scale=1.0)

        # ---- Stage C: v[j, h] = Vin^T-blocks @ Wv (bias folded in at the end) ----
        vpool = top.enter_context(tc.tile_pool(name="vres", bufs=1))
        vsb = vpool.tile([P, JT, H], BF16)
        with tc.tile_pool(name="sc1", bufs=1) as sc1, \
             tc.tile_pool(name="sc2", bufs=3) as sc2, \
             tc.tile_pool(name="sc3", bufs=3) as sc3, \
             tc.tile_pool(name="tpsC", bufs=4, space="PSUM") as tpsC, \
             tc.tile_pool(name="ppsC", bufs=2, space="PSUM") as ppsC:
            wvs = sc1.tile([P, D2T, H], F32R)
            nc.sync.dma_start(wvs[:], wv.rearrange("(t p) h -> p t h", p=P))
            for jt in range(JT):
                vrow = sc2.tile([P, D2], F32, tag="vrow")
                nc.sync.dma_start(vrow[:], vv[ts(jt, P), :])
                vT = sc3.tile([P, D2T, P], F32R, tag="vT")
                for dt in range(D2T):
                    pst = tpsC.tile([P, P], F32, tag="pst")
                    nc.tensor.transpose(pst[:], vrow[:, ts(dt, P)], ident[:])
                    nc.vector.tensor_copy(vT[:, dt, :], pst[:])
                for hc in range(H // 512):
                    psv = ppsC.tile([P, 512], F32, tag="psv")
                    for dt in range(D2T):
                        nc.tensor.matmul(psv[:], vT[:, dt, :],
                                         wvs[:, dt, ts(hc, 512)],
                                         start=(dt == 0), stop=(dt == D2T - 1))
                    nc.vector.tensor_copy(vsb[:, jt, ts(hc, 512)], psv[:])

        # ---- Stage D: per m-tile scores -> softmax -> (probs^T) @ v ----
        with tc.tile_pool(name="sd2", bufs=2) as sd2, \
             tc.tile_pool(name="sd3", bufs=2) as sd3, \
             tc.tile_pool(name="stat", bufs=3) as stat, \
             tc.tile_pool(name="sps", bufs=3, space="PSUM") as sps, \
             tc.tile_pool(name="tbf", bufs=2, space="PSUM") as tbf, \
             tc.tile_pool(name="avp", bufs=2, space="PSUM") as avp:
            for mt in range(MT):
                ssb = sd2.tile([P, JC, 512], F32, tag="ssb")
                mx4 = stat.tile([P, JC], F32, tag="mx4")
                for jc in range(JC):
                    pss = sps.tile([P, 512], F32, tag="pss")
                    for ht in range(HT):
                        nc.tensor.matmul(pss[:], qT[:, ht, ts(mt, P)],
                                         kT[:, ht, ts(jc, 512)],
                                         start=(ht == 0), stop=(ht == HT - 1))
                    nc.vector.tensor_copy(ssb[:, jc, :], pss[:])
                    nc.vector.reduce_max(mx4[:, jc:jc + 1], pss[:], axis=AX)
                negmax = stat.tile([P, 1], F32, tag="negmax")
                nc.vector.reduce_max(negmax[:], mx4[:], axis=AX, negate=True)
                wsb = sd2.tile([P, JC, 512], BF16, tag="wsb")
                sm4 = stat.tile([P, JC], F32, tag="sm4")
                for jc in range(JC):
                    nc.scalar.activation(wsb[:, jc, :], ssb[:, jc, :], AF.Exp,
                                         bias=negmax[:, 0:1], scale=1.0,
                                         accum_out=sm4[:, jc:jc + 1])
                ssum = stat.tile([P, 1], F32, tag="ssum")
                nc.vector.reduce_sum(ssum[:], sm4[:], axis=AX)
                rinv = stat.tile([P, 1], F32, tag="rinv")
                nc.vector.reciprocal(rinv[:], ssum[:])
                wT = sd3.tile([P, JT, P], BF16, tag="wT")
                for jt in range(JT):
                    pstb = tbf.tile([P, P], BF16, tag="pstb")
                    nc.tensor.transpose(pstb[:], wsb[:, jt // 4, ts(jt % 4, P)],
                                        identb[:])
                    nc.vector.tensor_copy(wT[:, jt, :], pstb[:])
                osb = sd2.tile([P, H], F32, tag="osb")
                for hc in range(H // 512):
                    psa = avp.tile([P, 512], F32, tag="psa")
                    for jt in range(JT):
                        nc.tensor.matmul(psa[:], wT[:, jt, :],
                                         vsb[:, jt, ts(hc, 512)],
                                         start=(jt == 0), stop=(jt == JT - 1))
                    nc.scalar.activation(osb[:, ts(hc, 512)], psa[:], AF.Copy,
                                         scale=rinv[:, 0:1])
                nc.vector.tensor_tensor(osb[:], osb[:], bv_full[:], ALU.add)
                nc.sync.dma_start(out[ts(mt, P), :], osb[:])

    nc.compile()
    return nc


def _get_nc():
    if "nc" not in _CACHE:
        _CACHE["nc"] = _build_bass()
    return _CACHE["nc"]


def kernel(query, key, value, Wq, bq, Wk, bk, Wv, bv):
    global LAST_RESULTS
    nc = _get_nc()

    def f(a):
        return np.ascontiguousarray(np.asarray(a, dtype=np.float32))

    query, key, value = f(query), f(key), f(value)
    Wq, bq, Wk, bk, Wv, bv = f(Wq), f(bq), f(Wk), f(bk), f(Wv), f(bv)

    in_maps = []
    half = LQ // 2
    for c in range(N_CORES):
        b, h = divmod(c, 2)
        in_maps.append({
            "xq": np.ascontiguousarray(query[b, h * half:(h + 1) * half, :]),
            "ky": key[b],
            "vv": value[b],
            "wq": Wq, "wk": Wk, "wv": Wv,
            "bq": bq, "bk": bk, "bv": bv,
        })

    res = run_bass_kernel_spmd(nc, in_maps, core_ids=list(range(N_CORES)))
    LAST_RESULTS = res

    out = np.empty((B, LQ, H), dtype=np.float32)
    for c in range(N_CORES):
        b, h = divmod(c, 2)
        out[b, h * half:(h + 1) * half, :] = res.results[c]["out"]
    return out
